# revision 26
# baseline (speedup 1.0000x reference)
"""Channel-attention (nn_ChannelAttentionModule) Trainium2 kernel.

Math (per batch b):
    X = x[b]  [C, N]  with C=512, N=64*64=4096
    q = Wq X + bq ; k = Wk X + bk ; v = Wv X + bv
    L = q k^T ; A = softmax(L, -1) ; out = A v + X

Restructure 1 (logits): L = Wq G Wk^T + u1 bk^T + bq (Wk S + N bk)^T with
G = X X^T (fp16 pass, block-upper-triangle, mirrored via f32r transposes),
S = X 1 harvested INSIDE the G pass via width-1 ones-matmuls into unused
PSUM columns, u1 = Wq S.  G Wk^T and Wq T1 are single-pass fp32r matmuls
(1 cyc/row at >=256-wide, ~2^-12 effective rounding).

Restructure 2 (v path): out = (A Wv + I) X + (A bv) 1^T — removes the whole
Wv X conv (C^2 N MACs) for the C^3 product A Wv; the residual rides the
diagonal of MT and A bv rides the PSUM-evacuation bias.

DMA: one shared transfer device services all queues; everything bulk is
host-prearranged so each transfer is partition-contiguous (fewer, fatter
descriptors), split across the SP/ACT HWDGE queues + Pool SWDGE lane.

Sharding: pure data-parallel, one batch per NeuronCore (B=8, 8 cores).
"""

import numpy as np

import concourse.mybir as mybir
import concourse.tile as tile
from concourse import bacc
from concourse.bass_utils import run_bass_kernel_spmd

F32 = mybir.dt.float32
F32R = mybir.dt.float32r
F16 = mybir.dt.float16
AX = mybir.AxisListType.X
EXP = mybir.ActivationFunctionType.Exp

B = 8
C = 512
HW = 64 * 64
P = 128
CH = C // P  # 4 channel chunks
NG = 8  # xtr granules (4 spatial tiles of 512 each)
# upper-triangle start per G row chunk (tight block triangle)
USTART = [0, 128, 256, 384]
# S_c accumulates in its own PSUM bank (pb0-3, free during the G phase):
# a start=True matmul clears has_written for the WHOLE bank, so every
# interleaved accumulation group must own a bank exclusively.


def _body(tc, nc, io):
    xt16, x16 = io["xt16"], io["x16"]
    wqt, wkt, wv16 = io["wqt"], io["wkt"], io["wv16"]
    bqr, bkr, nbkr, bv16 = io["bqr"], io["bkr"], io["nbkr"], io["bv16"]
    id16, id32, out = io["id16"], io["id32"], io["out16"]

    ps = tc.alloc_tile_pool(name="ps", bufs=1, space="PSUM")
    sb = tc.alloc_tile_pool(name="sb", bufs=1)
    so = tc.alloc_tile_pool(name="so", bufs=2)

    # ---- persistent SBUF tiles ----
    x16_sb = [sb.tile([P, HW], F16, name=f"x16_{i}", tag=f"x16_{i}") for i in range(CH)]
    ar_sb = [sb.tile([P, 4 * C], F16, name=f"ar{g}", tag=f"ar{g}") for g in range(NG)]
    wqt_sb = sb.tile([P, CH * C], F32R, name="wqt_sb", tag="wqt_sb")
    wkt_sb = sb.tile([P, CH * C], F32R, name="wkt_sb", tag="wkt_sb")
    wv_sb = sb.tile([P, CH * C], F16, name="wv_sb", tag="wv_sb")
    id16_sb = sb.tile([P, P], F16, name="id16sb", tag="id16sb")
    id32_sb = sb.tile([P, P], F32R, name="id32sb", tag="id32sb")
    bv_sb = sb.tile([P, CH], F16, name="bv_sb", tag="bv_sb")
    nbkr_sb = sb.tile([1, C], F32, name="nbkrsb", tag="nbkrsb")
    bq_sb = sb.tile([1, C], F32R, name="bq_sb", tag="bq_sb")
    bk_sb = sb.tile([1, C], F32R, name="bk_sb", tag="bk_sb")
    ones16 = sb.tile([P, 1], F16, name="ones16", tag="ones16")
    nc.gpsimd.memset(ones16, 1.0)


    def xtr_load(g2, q):
        q.dma_start(ar_sb[g2], xt16[g2 * P : (g2 + 1) * P, :])

    g_ps = [ps.tile([P, C], F32, name=f"gps{i}", tag=f"pa{i}") for i in range(CH)]
    s_ps = [ps.tile([P, 1], F32, name=f"sps{i}", tag=f"pb{i}") for i in range(CH)]

    def g_pass(g2):
        ar4 = ar_sb[g2]
        for t in range(4):
            n = g2 * 4 + t
            first, last = n == 0, n == 4 * NG - 1
            for c in range(CH):
                u = USTART[c]
                nc.tensor.matmul(
                    g_ps[c][:, u:],
                    lhsT=ar4[:, t * C + c * P : t * C + (c + 1) * P],
                    rhs=ar4[:, t * C + u : (t + 1) * C],
                    start=first,
                    stop=last,
                )
            for c in range(CH):
                nc.tensor.matmul(
                    s_ps[c],
                    lhsT=ar4[:, t * C + c * P : t * C + (c + 1) * P],
                    rhs=ones16,
                    start=first,
                    stop=last,
                )

    def x16_load(c, h, q):
        q.dma_start(
            x16_sb[c][:, h * 2048 : (h + 1) * 2048],
            x16[c * P : (c + 1) * P, h * 2048 : (h + 1) * 2048],
        )

    # ---- front DMA schedule ----
    # sync(SP):   g0 halves, g2, g4, g6, wqt, x16 c0/c1
    # scalar(ACT):g1, g3, g5, g7, wkt, x16 c2/c3
    # gpsimd(Pool/SWDGE lane): id32, bqr, bkr, nbkr, id16, bv16, wv
    nc.sync.dma_start(ar_sb[0][:, 0:C], xt16[0:P, 0:C])
    nc.sync.dma_start(ar_sb[0][:, C : 2 * C], xt16[0:P, C : 2 * C])
    nc.sync.dma_start(ar_sb[0][:, 2 * C :], xt16[0:P, 2 * C :])
    nc.scalar.dma_start(ar_sb[1][:, 0 : 2 * C], xt16[P : 2 * P, 0 : 2 * C])
    nc.scalar.dma_start(ar_sb[1][:, 2 * C :], xt16[P : 2 * P, 2 * C :])
    nc.gpsimd.dma_start(id32_sb, id32)
    xtr_load(2, nc.gpsimd)
    xtr_load(3, nc.scalar)
    xtr_load(4, nc.sync)
    g_pass(0)
    nc.gpsimd.dma_start(bq_sb, bqr)
    nc.gpsimd.dma_start(bk_sb, bkr)
    xtr_load(6, nc.sync)
    xtr_load(5, nc.scalar)
    nc.gpsimd.dma_start(nbkr_sb, nbkr)
    g_pass(1)
    xtr_load(7, nc.scalar)
    nc.gpsimd.dma_start(id16_sb, id16)
    nc.gpsimd.dma_start(bv_sb, bv16)
    g_pass(2)
    nc.sync.dma_start(wqt_sb, wqt)
    nc.scalar.dma_start(wkt_sb, wkt)
    nc.gpsimd.dma_start(wv_sb, wv16)
    g_pass(3)
    x16_load(0, 0, nc.sync)
    x16_load(2, 0, nc.scalar)
    g_pass(4)
    x16_load(0, 1, nc.sync)
    x16_load(2, 1, nc.scalar)
    g_pass(5)
    x16_load(1, 0, nc.sync)
    x16_load(3, 0, nc.scalar)
    g_pass(6)
    x16_load(1, 1, nc.sync)
    x16_load(3, 1, nc.scalar)
    g_pass(7)

    # ---- S -> f32r (tiny copies from the G-pass PSUM columns) ----
    s32r = [sb.tile([P, 1], F32R, name=f"sr{i}", tag=f"sr{i}") for i in range(CH)]
    for i in (0, 1):
        nc.scalar.copy(s32r[i], s_ps[i])
    for i in (2, 3):
        nc.vector.tensor_copy(s32r[i], s_ps[i])

    # ---- u1 = Wq S, u2 = Wk S (fp32r): fills the PE gap while G stages ----
    u1_ps = ps.tile([1, C], F32, name="u1ps", tag="pb2")
    u2_ps = ps.tile([1, C], F32, name="u2ps", tag="pb3")
    for e in range(CH):
        nc.tensor.matmul(
            u1_ps, lhsT=s32r[e], rhs=wqt_sb[:, e * C : (e + 1) * C],
            start=e == 0, stop=e == CH - 1,
        )
    for e in range(CH):
        nc.tensor.matmul(
            u2_ps, lhsT=s32r[e], rhs=wkt_sb[:, e * C : (e + 1) * C],
            start=e == 0, stop=e == CH - 1,
        )

    # ---- stage G -> f32r SBUF (split ACT/DVE), mirror via f32r transposes ----
    g_sb = [sb.tile([P, C], F32R, name=f"gsb{i}", tag=f"gsb{i}") for i in range(CH)]
    for c in range(CH):
        for blk in range(USTART[c] // P, CH):
            dst = g_sb[c][:, blk * P : (blk + 1) * P]
            src_ = g_ps[c][:, blk * P : (blk + 1) * P]
            if (c + blk) % 2 == 0:
                nc.scalar.copy(dst, src_)
            else:
                nc.vector.tensor_copy(dst, src_)
    mi = 0
    for c in range(1, CH):
        for d in range(c):
            tp = ps.tile([P, P], F32, name=f"mtp{c}{d}", tag=f"pb{mi % 2}")
            nc.tensor.transpose(
                tp.bitcast(F32R), g_sb[d][:, c * P : (c + 1) * P], id32_sb
            )
            if mi % 2 == 0:
                nc.scalar.copy(g_sb[c][:, d * P : (d + 1) * P], tp)
            else:
                nc.vector.tensor_copy(g_sb[c][:, d * P : (d + 1) * P], tp)
            mi += 1

    # ---- rank-1 operand rows (all partition-0, no DMA) ----
    rhs2f = sb.tile([1, C], F32, name="rhs2f", tag="rhs2f")
    nc.vector.tensor_add(rhs2f, u2_ps, nbkr_sb)
    u2r = sb.tile([1, C], F32R, name="u2r", tag="u2r")
    nc.scalar.copy(u2r, rhs2f)
    u1r = sb.tile([1, C], F32R, name="u1r", tag="u1r")
    nc.scalar.copy(u1r, u1_ps)

    # ---- T1 = G Wk^T, single-pass fp32r ----
    t1_ps = [ps.tile([P, C], F32, name=f"t1ps{i}", tag=f"pa{i}") for i in range(CH)]
    for f in range(CH):
        for e in range(CH):
            nc.tensor.matmul(
                t1_ps[e],
                lhsT=g_sb[f][:, e * P : (e + 1) * P],
                rhs=wkt_sb[:, f * C : (f + 1) * C],
                start=f == 0,
                stop=f == CH - 1,
            )

    # ---- stage T1 -> f32r SBUF (split ACT/DVE) ----
    t1_sb = [sb.tile([P, C], F32R, name=f"t1sb{i}", tag=f"t1sb{i}") for i in range(CH)]
    for e in range(CH):
        if e % 2 == 0:
            nc.scalar.copy(t1_sb[e], t1_ps[e])
        else:
            nc.vector.tensor_copy(t1_sb[e], t1_ps[e])

    # ---- logits = Wq T1 (fp32r), then the rank-1 terms ----
    l_ps = [ps.tile([P, C], F32, name=f"lps{i}", tag=f"pb{i}") for i in range(CH)]
    for c in range(CH):
        for e in range(CH):
            nc.tensor.matmul(
                l_ps[c],
                lhsT=wqt_sb[:, e * C + c * P : e * C + (c + 1) * P],
                rhs=t1_sb[e],
                start=e == 0,
                stop=False,
            )
        nc.tensor.matmul(
            l_ps[c], lhsT=bq_sb[0:1, c * P : (c + 1) * P], rhs=u2r,
            start=False, stop=False,
        )
        nc.tensor.matmul(
            l_ps[c], lhsT=u1r[0:1, c * P : (c + 1) * P], rhs=bk_sb,
            start=False, stop=True,
        )

    # ---- softmax numerator E = exp(L - max) in fp16; normalization is
    #      deferred: rcp rides the out-evacuation scale, and the residual is
    #      folded as diag(ssum) into MT_E (rcp*ssum == 1 restores +X) ----
    e16_sb = [sb.tile([P, C], F16, name=f"e16_{i}", tag=f"e16_{i}") for i in range(CH)]
    rcp = [sb.tile([P, 1], F32, name=f"rcp{c}", tag=f"rcp{c}") for c in range(CH)]
    diagm = [sb.tile([P, P], F16, name=f"dgm{c}", tag=f"dgm{c}") for c in range(CH)]
    for c in range(CH):
        negmx = sb.tile([P, 1], F32, name=f"negmx{c}", tag=f"negmx{c}")
        nc.vector.reduce_max(negmx, l_ps[c], axis=AX, negate=True)
        ssum = sb.tile([P, 1], F32, name=f"ssum{c}", tag=f"ssum{c}")
        nc.scalar.activation(
            e16_sb[c], l_ps[c], EXP, bias=negmx, scale=1.0, accum_out=ssum
        )
        nc.vector.reciprocal(rcp[c], ssum)
        nc.vector.tensor_scalar_mul(diagm[c], id16_sb, ssum)

    # ---- transpose E (fp16, PE, i-outer): wt_sb[j] = E^T chunk j ----
    wt_sb = [sb.tile([P, C], F16, name=f"wtsb{j}", tag=f"wtsb{j}") for j in range(CH)]
    wt_ps = [ps.tile([P, C], F16, name=f"wtps{j}", tag=f"pa{j}") for j in range(CH)]
    for i in range(CH):
        for j in range(CH):
            nc.tensor.transpose(
                wt_ps[j][:, i * P : (i + 1) * P],
                e16_sb[i][:, j * P : (j + 1) * P],
                id16_sb,
            )
    for j in range(CH):
        if j % 2 == 0:
            nc.vector.tensor_copy(wt_sb[j], wt_ps[j])
        else:
            nc.scalar.copy(wt_sb[j], wt_ps[j])

    # ---- r~ = rcp * (E bv) (tiny matmuls + elementwise) ----
    r_col = [sb.tile([P, 1], F32, name=f"rcol{i}", tag=f"rcol{i}") for i in range(CH)]
    for ic in range(CH):
        r_ps = ps.tile([P, 1], F32, name=f"rps{ic}", tag=f"pb{ic}")
        for oc in range(CH):
            nc.tensor.matmul(
                r_ps,
                lhsT=wt_sb[oc][:, ic * P : (ic + 1) * P],
                rhs=bv_sb[:, oc : oc + 1],
                start=oc == 0,
                stop=oc == CH - 1,
            )
        nc.vector.tensor_mul(r_col[ic], r_ps, rcp[ic])

    # ---- MT = (A Wv)^T + I fold (fp16, evac split ACT/DVE) ----
    mt_sb = [sb.tile([P, C], F16, name=f"mtsb{i}", tag=f"mtsb{i}") for i in range(CH)]
    for cc in range(CH):
        mt_ps = ps.tile([P, C], F32, name=f"mtps{cc}", tag=f"pa{cc}")
        for oc in range(CH):
            nc.tensor.matmul(
                mt_ps,
                lhsT=wv_sb[:, oc * C + cc * P : oc * C + (cc + 1) * P],
                rhs=wt_sb[oc],
                start=oc == 0,
                stop=oc == CH - 1,
            )
        for blk in range(CH):
            dst = mt_sb[cc][:, blk * P : (blk + 1) * P]
            src_ = mt_ps[:, blk * P : (blk + 1) * P]
            if blk == cc:
                nc.vector.tensor_add(dst, src_, diagm[cc])
            elif (blk + cc) % 2 == 0:
                nc.scalar.copy(dst, src_)
            else:
                nc.vector.tensor_copy(dst, src_)

    # ---- out = MT'^T X + r; evac alternates ACT/DVE; contiguous DMA per nt.
    #      The last tile is split into 256-col halves with per-piece DMAs so
    #      the final evacuation+writeback drain is short. ----
    def out_evac(o_slice, o_ps, ic, eng):
        if eng == 0:
            nc.scalar.activation(
                o_slice, o_ps, mybir.ActivationFunctionType.Identity,
                bias=r_col[ic], scale=rcp[ic],
            )
        else:
            nc.vector.tensor_scalar(
                o_slice, o_ps, rcp[ic], r_col[ic],
                mybir.AluOpType.mult, mybir.AluOpType.add,
            )

    for nt in range(NG - 1):
        bank = "pb" if nt % 2 == 0 else "pa"
        o_big = so.tile([P, CH * 512], F16, name="obig", tag="obig", bufs=2)
        for ic in range(CH):
            o_ps = ps.tile([P, 512], F32, name=f"ops{ic}", tag=f"{bank}{ic}")
            for cc in range(CH):
                nc.tensor.matmul(
                    o_ps,
                    lhsT=mt_sb[cc][:, ic * P : (ic + 1) * P],
                    rhs=x16_sb[cc][:, nt * 512 : (nt + 1) * 512],
                    start=cc == 0,
                    stop=cc == CH - 1,
                )
            out_evac(o_big[:, ic * 512 : (ic + 1) * 512], o_ps, ic, (nt + ic) % 2)
        q = nc.sync if nt % 2 == 0 else nc.scalar
        q.dma_start(out[:, nt * CH * 512 : (nt + 1) * CH * 512], o_big)

    nt = NG - 1
    o_big = so.tile([P, CH * 512], F16, name="obig", tag="obig", bufs=2)
    qs = [nc.sync, nc.scalar, nc.gpsimd, nc.sync]
    for ic in range(CH):
        for half in range(2):
            bank = "pa" if half == 0 else "pb"
            o_ps = ps.tile([P, 256], F32, name=f"opsl{ic}{half}", tag=f"{bank}{ic}")
            nlo = nt * 512 + half * 256
            for cc in range(CH):
                nc.tensor.matmul(
                    o_ps,
                    lhsT=mt_sb[cc][:, ic * P : (ic + 1) * P],
                    rhs=x16_sb[cc][:, nlo : nlo + 256],
                    start=cc == 0,
                    stop=cc == CH - 1,
                )
            out_evac(
                o_big[:, ic * 512 + half * 256 : ic * 512 + half * 256 + 256],
                o_ps, ic, (ic + half) % 2,
            )
        qs[ic].dma_start(
            out[:, nt * CH * 512 + ic * 512 : nt * CH * 512 + (ic + 1) * 512],
            o_big[:, ic * 512 : (ic + 1) * 512],
        )

    for pool in (so, sb, ps):
        pool.release()


def _build_nc(repeat=1):
    nc = bacc.Bacc(
        "TRN2",
        target_bir_lowering=False,
        debug=False,
        num_devices=B,
        enable_asserts=False,
    )
    io = {}
    dt = nc.dram_tensor
    # xt16: granule-contiguous X^T blob: row (g*P+p) = 4 tiles' channel rows
    io["xt16"] = dt("xt16", (NG * P, 4 * C), F16, kind="ExternalInput").ap()
    io["x16"] = dt("x16", (C, HW), F16, kind="ExternalInput").ap()
    # weights prearranged [P, CH*C] so each is one contiguous DMA
    io["wqt"] = dt("wqt", (P, CH * C), F32R, kind="ExternalInput").ap()
    io["wkt"] = dt("wkt", (P, CH * C), F32R, kind="ExternalInput").ap()
    io["wv16"] = dt("wv16", (P, CH * C), F16, kind="ExternalInput").ap()
    io["bqr"] = dt("bqr", (1, C), F32R, kind="ExternalInput").ap()
    io["bkr"] = dt("bkr", (1, C), F32R, kind="ExternalInput").ap()
    io["nbkr"] = dt("nbkr", (1, C), F32, kind="ExternalInput").ap()
    io["bv16"] = dt("bv16", (P, CH), F16, kind="ExternalInput").ap()
    io["id16"] = dt("id16", (P, P), F16, kind="ExternalInput").ap()
    io["id32"] = dt("id32", (P, P), F32R, kind="ExternalInput").ap()
    # out: [P, NT*CH*512] f16, host unscrambles
    io["out16"] = dt("out16", (P, NG * CH * 512), F16, kind="ExternalOutput").ap()
    with tile.TileContext(nc) as tc:
        for _ in range(repeat):
            _body(tc, nc, io)
    nc.compile()
    return nc


_NC_CACHE = None


def get_nc():
    global _NC_CACHE
    if _NC_CACHE is None:
        _NC_CACHE = _build_nc()
    return _NC_CACHE


def prep_in_maps(x, wq, bq, wk, bk, wv, bv):
    """Host-side input prep: reshape/transpose/dtype casts only."""
    x = np.asarray(x, dtype=np.float32)
    X = x.reshape(B, C, HW)
    XT = X.transpose(0, 2, 1)  # [B, N, C]
    # granule-contiguous blob: [B, NG*P, 4*C]
    xt16 = np.ascontiguousarray(
        XT.reshape(B, NG, 4, P, C).transpose(0, 1, 3, 2, 4).reshape(B, NG * P, 4 * C)
    ).astype(np.float16)
    x16 = X.astype(np.float16)

    def warr(w):  # [C, C] -> w^T chunked [P, CH*C]
        wt = np.asarray(w, np.float32).T  # [c, o]
        return np.ascontiguousarray(
            wt.reshape(CH, P, C).transpose(1, 0, 2).reshape(P, CH * C)
        )

    wqta = warr(wq)
    wkta = warr(wk)
    wv16a = np.ascontiguousarray(
        np.asarray(wv, np.float32).reshape(CH, P, C).transpose(1, 0, 2).reshape(P, CH * C)
    ).astype(np.float16)
    bqr = np.asarray(bq, np.float32).reshape(1, C)
    bkr = np.asarray(bk, np.float32).reshape(1, C)
    nbkr = (float(HW) * np.asarray(bk, np.float32)).reshape(1, C)
    bv16 = np.ascontiguousarray(
        np.asarray(bv, np.float32).reshape(CH, P).T
    ).astype(np.float16)
    id16 = np.eye(P, dtype=np.float16)
    id32 = np.eye(P, dtype=np.float32)
    in_maps = []
    for b in range(B):
        in_maps.append(
            {
                "xt16": xt16[b],
                "x16": np.ascontiguousarray(x16[b]),
                "wqt": wqta,
                "wkt": wkta,
                "wv16": wv16a,
                "bqr": bqr,
                "bkr": bkr,
                "nbkr": nbkr,
                "bv16": bv16,
                "id16": id16,
                "id32": id32,
            }
        )
    return in_maps


def kernel(x, wq, bq, wk, bk, wv, bv):
    nc = get_nc()
    in_maps = prep_in_maps(x, wq, bq, wk, bk, wv, bv)
    res = run_bass_kernel_spmd(nc, in_maps, core_ids=list(range(B)))
    # out16 blob [P, NG*CH*512]: [p, nt, e, n_local] -> out[e*P+p, nt*512+n]
    blobs = np.stack([res.results[b]["out16"] for b in range(B)])
    outs = blobs.reshape(B, P, NG, CH, 512).transpose(0, 3, 1, 2, 4).reshape(B, C, HW)
    return outs.reshape(B, C, 64, 64).astype(np.float32)


# revision 27
# speedup vs baseline: 1.0015x; 1.0015x over previous
"""Channel-attention (nn_ChannelAttentionModule) Trainium2 kernel.

Math (per batch b):
    X = x[b]  [C, N]  with C=512, N=64*64=4096
    q = Wq X + bq ; k = Wk X + bk ; v = Wv X + bv
    L = q k^T ; A = softmax(L, -1) ; out = A v + X

Restructure 1 (logits): L = Wq G Wk^T + u1 bk^T + bq (Wk S + N bk)^T with
G = X X^T (fp16 pass, block-upper-triangle, mirrored via f32r transposes),
S = X 1 harvested INSIDE the G pass via width-1 ones-matmuls into unused
PSUM columns, u1 = Wq S.  G Wk^T and Wq T1 are single-pass fp32r matmuls
(1 cyc/row at >=256-wide, ~2^-12 effective rounding).

Restructure 2 (v path): out = (A Wv + I) X + (A bv) 1^T — removes the whole
Wv X conv (C^2 N MACs) for the C^3 product A Wv; the residual rides the
diagonal of MT and A bv rides the PSUM-evacuation bias.

DMA: one shared transfer device services all queues; everything bulk is
host-prearranged so each transfer is partition-contiguous (fewer, fatter
descriptors), split across the SP/ACT HWDGE queues + Pool SWDGE lane.

Sharding: pure data-parallel, one batch per NeuronCore (B=8, 8 cores).
"""

import numpy as np

import concourse.mybir as mybir
import concourse.tile as tile
from concourse import bacc
from concourse.bass_utils import run_bass_kernel_spmd

F32 = mybir.dt.float32
F32R = mybir.dt.float32r
F16 = mybir.dt.float16
AX = mybir.AxisListType.X
EXP = mybir.ActivationFunctionType.Exp

B = 8
C = 512
HW = 64 * 64
P = 128
CH = C // P  # 4 channel chunks
NG = 8  # xtr granules (4 spatial tiles of 512 each)
# upper-triangle start per G row chunk (tight block triangle)
USTART = [0, 128, 256, 384]
# S_c accumulates in its own PSUM bank (pb0-3, free during the G phase):
# a start=True matmul clears has_written for the WHOLE bank, so every
# interleaved accumulation group must own a bank exclusively.


def _body(tc, nc, io):
    xt16, x16 = io["xt16"], io["x16"]
    wqt, wkt, wv16 = io["wqt"], io["wkt"], io["wv16"]
    bqr, bkr, nbkr, bv16 = io["bqr"], io["bkr"], io["nbkr"], io["bv16"]
    id16, id32, out = io["id16"], io["id32"], io["out16"]

    ps = tc.alloc_tile_pool(name="ps", bufs=1, space="PSUM")
    sb = tc.alloc_tile_pool(name="sb", bufs=1)
    so = tc.alloc_tile_pool(name="so", bufs=2)

    # ---- persistent SBUF tiles ----
    x16_sb = [sb.tile([P, HW], F16, name=f"x16_{i}", tag=f"x16_{i}") for i in range(CH)]
    ar_sb = [sb.tile([P, 4 * C], F16, name=f"ar{g}", tag=f"ar{g}") for g in range(NG)]
    wqt_sb = sb.tile([P, CH * C], F32R, name="wqt_sb", tag="wqt_sb")
    wkt_sb = sb.tile([P, CH * C], F32R, name="wkt_sb", tag="wkt_sb")
    wv_sb = sb.tile([P, CH * C], F16, name="wv_sb", tag="wv_sb")
    id16_sb = sb.tile([P, P], F16, name="id16sb", tag="id16sb")
    id32_sb = sb.tile([P, P], F32R, name="id32sb", tag="id32sb")
    bv_sb = sb.tile([P, CH], F16, name="bv_sb", tag="bv_sb")
    nbkr_sb = sb.tile([1, C], F32, name="nbkrsb", tag="nbkrsb")
    bq_sb = sb.tile([1, C], F32R, name="bq_sb", tag="bq_sb")
    bk_sb = sb.tile([1, C], F32R, name="bk_sb", tag="bk_sb")
    ones16 = sb.tile([P, 1], F16, name="ones16", tag="ones16")
    nc.gpsimd.memset(ones16, 1.0)


    def xtr_load(g2, q):
        q.dma_start(ar_sb[g2], xt16[g2 * P : (g2 + 1) * P, :])

    g_ps = [ps.tile([P, C], F32, name=f"gps{i}", tag=f"pa{i}") for i in range(CH)]
    s_ps = [ps.tile([P, 1], F32, name=f"sps{i}", tag=f"pb{i}") for i in range(CH)]

    def g_pass(g2):
        ar4 = ar_sb[g2]
        for t in range(4):
            n = g2 * 4 + t
            first, last = n == 0, n == 4 * NG - 1
            for c in range(CH):
                u = USTART[c]
                nc.tensor.matmul(
                    g_ps[c][:, u:],
                    lhsT=ar4[:, t * C + c * P : t * C + (c + 1) * P],
                    rhs=ar4[:, t * C + u : (t + 1) * C],
                    start=first,
                    stop=last,
                )
            for c in range(CH):
                nc.tensor.matmul(
                    s_ps[c],
                    lhsT=ar4[:, t * C + c * P : t * C + (c + 1) * P],
                    rhs=ones16,
                    start=first,
                    stop=last,
                )

    def x16_load(c, h, q):
        q.dma_start(
            x16_sb[c][:, h * 2048 : (h + 1) * 2048],
            x16[c * P : (c + 1) * P, h * 2048 : (h + 1) * 2048],
        )

    # ---- front DMA schedule ----
    # sync(SP):   g0 halves, g2, g4, g6, wqt, x16 c0/c1
    # scalar(ACT):g1, g3, g5, g7, wkt, x16 c2/c3
    # gpsimd(Pool/SWDGE lane): id32, bqr, bkr, nbkr, id16, bv16, wv
    nc.sync.dma_start(ar_sb[0][:, 0:C], xt16[0:P, 0:C])
    nc.sync.dma_start(ar_sb[0][:, C : 2 * C], xt16[0:P, C : 2 * C])
    nc.sync.dma_start(ar_sb[0][:, 2 * C :], xt16[0:P, 2 * C :])
    nc.scalar.dma_start(ar_sb[1][:, 0 : 2 * C], xt16[P : 2 * P, 0 : 2 * C])
    nc.scalar.dma_start(ar_sb[1][:, 2 * C :], xt16[P : 2 * P, 2 * C :])
    nc.gpsimd.dma_start(id32_sb, id32)
    xtr_load(2, nc.sync)
    xtr_load(3, nc.scalar)
    nc.gpsimd.dma_start(bq_sb, bqr)
    nc.gpsimd.dma_start(bk_sb, bkr)
    g_pass(0)
    xtr_load(6, nc.gpsimd)
    xtr_load(4, nc.sync)
    xtr_load(5, nc.scalar)
    nc.gpsimd.dma_start(nbkr_sb, nbkr)
    g_pass(1)
    xtr_load(7, nc.scalar)
    nc.gpsimd.dma_start(id16_sb, id16)
    nc.gpsimd.dma_start(bv_sb, bv16)
    g_pass(2)
    nc.sync.dma_start(wqt_sb, wqt)
    nc.scalar.dma_start(wkt_sb, wkt)
    nc.gpsimd.dma_start(wv_sb, wv16)
    g_pass(3)
    x16_load(0, 0, nc.sync)
    x16_load(2, 0, nc.scalar)
    g_pass(4)
    x16_load(0, 1, nc.sync)
    x16_load(2, 1, nc.scalar)
    g_pass(5)
    x16_load(1, 0, nc.sync)
    x16_load(3, 0, nc.scalar)
    g_pass(6)
    x16_load(1, 1, nc.sync)
    x16_load(3, 1, nc.scalar)
    g_pass(7)

    # ---- S -> f32r (tiny copies from the G-pass PSUM columns) ----
    s32r = [sb.tile([P, 1], F32R, name=f"sr{i}", tag=f"sr{i}") for i in range(CH)]
    for i in (0, 1):
        nc.scalar.copy(s32r[i], s_ps[i])
    for i in (2, 3):
        nc.vector.tensor_copy(s32r[i], s_ps[i])

    # ---- u1 = Wq S, u2 = Wk S (fp32r): fills the PE gap while G stages ----
    u1_ps = ps.tile([1, C], F32, name="u1ps", tag="pb2")
    u2_ps = ps.tile([1, C], F32, name="u2ps", tag="pb3")
    for e in range(CH):
        nc.tensor.matmul(
            u1_ps, lhsT=s32r[e], rhs=wqt_sb[:, e * C : (e + 1) * C],
            start=e == 0, stop=e == CH - 1,
        )
    for e in range(CH):
        nc.tensor.matmul(
            u2_ps, lhsT=s32r[e], rhs=wkt_sb[:, e * C : (e + 1) * C],
            start=e == 0, stop=e == CH - 1,
        )

    # ---- stage G -> f32r SBUF (split ACT/DVE), mirror via f32r transposes ----
    g_sb = [sb.tile([P, C], F32R, name=f"gsb{i}", tag=f"gsb{i}") for i in range(CH)]
    for c in range(CH):
        for blk in range(USTART[c] // P, CH):
            dst = g_sb[c][:, blk * P : (blk + 1) * P]
            src_ = g_ps[c][:, blk * P : (blk + 1) * P]
            if (c + blk) % 2 == 0:
                nc.scalar.copy(dst, src_)
            else:
                nc.vector.tensor_copy(dst, src_)
    mi = 0
    for c in range(1, CH):
        for d in range(c):
            tp = ps.tile([P, P], F32, name=f"mtp{c}{d}", tag=f"pb{mi % 2}")
            nc.tensor.transpose(
                tp.bitcast(F32R), g_sb[d][:, c * P : (c + 1) * P], id32_sb
            )
            if mi % 2 == 0:
                nc.scalar.copy(g_sb[c][:, d * P : (d + 1) * P], tp)
            else:
                nc.vector.tensor_copy(g_sb[c][:, d * P : (d + 1) * P], tp)
            mi += 1

    # ---- rank-1 operand rows (all partition-0, no DMA) ----
    rhs2f = sb.tile([1, C], F32, name="rhs2f", tag="rhs2f")
    nc.vector.tensor_add(rhs2f, u2_ps, nbkr_sb)
    u2r = sb.tile([1, C], F32R, name="u2r", tag="u2r")
    nc.scalar.copy(u2r, rhs2f)
    u1r = sb.tile([1, C], F32R, name="u1r", tag="u1r")
    nc.scalar.copy(u1r, u1_ps)

    # ---- T1 = G Wk^T, single-pass fp32r ----
    t1_ps = [ps.tile([P, C], F32, name=f"t1ps{i}", tag=f"pa{i}") for i in range(CH)]
    for f in range(CH):
        for e in range(CH):
            nc.tensor.matmul(
                t1_ps[e],
                lhsT=g_sb[f][:, e * P : (e + 1) * P],
                rhs=wkt_sb[:, f * C : (f + 1) * C],
                start=f == 0,
                stop=f == CH - 1,
            )

    # ---- stage T1 -> f32r SBUF (split ACT/DVE) ----
    t1_sb = [sb.tile([P, C], F32R, name=f"t1sb{i}", tag=f"t1sb{i}") for i in range(CH)]
    for e in range(CH):
        if e % 2 == 0:
            nc.scalar.copy(t1_sb[e], t1_ps[e])
        else:
            nc.vector.tensor_copy(t1_sb[e], t1_ps[e])

    # ---- logits = Wq T1 (fp32r), then the rank-1 terms ----
    l_ps = [ps.tile([P, C], F32, name=f"lps{i}", tag=f"pb{i}") for i in range(CH)]
    for c in range(CH):
        for e in range(CH):
            nc.tensor.matmul(
                l_ps[c],
                lhsT=wqt_sb[:, e * C + c * P : e * C + (c + 1) * P],
                rhs=t1_sb[e],
                start=e == 0,
                stop=False,
            )
        nc.tensor.matmul(
            l_ps[c], lhsT=bq_sb[0:1, c * P : (c + 1) * P], rhs=u2r,
            start=False, stop=False,
        )
        nc.tensor.matmul(
            l_ps[c], lhsT=u1r[0:1, c * P : (c + 1) * P], rhs=bk_sb,
            start=False, stop=True,
        )

    # ---- softmax numerator E = exp(L - max) in fp16; normalization is
    #      deferred: rcp rides the out-evacuation scale, and the residual is
    #      folded as diag(ssum) into MT_E (rcp*ssum == 1 restores +X) ----
    e16_sb = [sb.tile([P, C], F16, name=f"e16_{i}", tag=f"e16_{i}") for i in range(CH)]
    rcp = [sb.tile([P, 1], F32, name=f"rcp{c}", tag=f"rcp{c}") for c in range(CH)]
    diagm = [sb.tile([P, P], F16, name=f"dgm{c}", tag=f"dgm{c}") for c in range(CH)]
    for c in range(CH):
        negmx = sb.tile([P, 1], F32, name=f"negmx{c}", tag=f"negmx{c}")
        nc.vector.reduce_max(negmx, l_ps[c], axis=AX, negate=True)
        ssum = sb.tile([P, 1], F32, name=f"ssum{c}", tag=f"ssum{c}")
        nc.scalar.activation(
            e16_sb[c], l_ps[c], EXP, bias=negmx, scale=1.0, accum_out=ssum
        )
        nc.vector.reciprocal(rcp[c], ssum)
        nc.vector.tensor_scalar_mul(diagm[c], id16_sb, ssum)

    # ---- transpose E (fp16, PE, i-outer): wt_sb[j] = E^T chunk j ----
    wt_sb = [sb.tile([P, C], F16, name=f"wtsb{j}", tag=f"wtsb{j}") for j in range(CH)]
    wt_ps = [ps.tile([P, C], F16, name=f"wtps{j}", tag=f"pa{j}") for j in range(CH)]
    for i in range(CH):
        for j in range(CH):
            nc.tensor.transpose(
                wt_ps[j][:, i * P : (i + 1) * P],
                e16_sb[i][:, j * P : (j + 1) * P],
                id16_sb,
            )
    for j in range(CH):
        if j % 2 == 0:
            nc.vector.tensor_copy(wt_sb[j], wt_ps[j])
        else:
            nc.scalar.copy(wt_sb[j], wt_ps[j])

    # ---- r~ = rcp * (E bv) (tiny matmuls + elementwise) ----
    r_col = [sb.tile([P, 1], F32, name=f"rcol{i}", tag=f"rcol{i}") for i in range(CH)]
    for ic in range(CH):
        r_ps = ps.tile([P, 1], F32, name=f"rps{ic}", tag=f"pb{ic}")
        for oc in range(CH):
            nc.tensor.matmul(
                r_ps,
                lhsT=wt_sb[oc][:, ic * P : (ic + 1) * P],
                rhs=bv_sb[:, oc : oc + 1],
                start=oc == 0,
                stop=oc == CH - 1,
            )
        nc.vector.tensor_mul(r_col[ic], r_ps, rcp[ic])

    # ---- MT = (A Wv)^T + I fold (fp16, evac split ACT/DVE) ----
    mt_sb = [sb.tile([P, C], F16, name=f"mtsb{i}", tag=f"mtsb{i}") for i in range(CH)]
    for cc in range(CH):
        mt_ps = ps.tile([P, C], F32, name=f"mtps{cc}", tag=f"pa{cc}")
        for oc in range(CH):
            nc.tensor.matmul(
                mt_ps,
                lhsT=wv_sb[:, oc * C + cc * P : oc * C + (cc + 1) * P],
                rhs=wt_sb[oc],
                start=oc == 0,
                stop=oc == CH - 1,
            )
        for blk in range(CH):
            dst = mt_sb[cc][:, blk * P : (blk + 1) * P]
            src_ = mt_ps[:, blk * P : (blk + 1) * P]
            if blk == cc:
                nc.vector.tensor_add(dst, src_, diagm[cc])
            elif (blk + cc) % 2 == 0:
                nc.scalar.copy(dst, src_)
            else:
                nc.vector.tensor_copy(dst, src_)

    # ---- out = MT'^T X + r; evac alternates ACT/DVE; contiguous DMA per nt.
    #      The last tile is split into 256-col halves with per-piece DMAs so
    #      the final evacuation+writeback drain is short. ----
    def out_evac(o_slice, o_ps, ic, eng):
        if eng == 0:
            nc.scalar.activation(
                o_slice, o_ps, mybir.ActivationFunctionType.Identity,
                bias=r_col[ic], scale=rcp[ic],
            )
        else:
            nc.vector.tensor_scalar(
                o_slice, o_ps, rcp[ic], r_col[ic],
                mybir.AluOpType.mult, mybir.AluOpType.add,
            )

    for nt in range(NG - 1):
        bank = "pb" if nt % 2 == 0 else "pa"
        o_big = so.tile([P, CH * 512], F16, name="obig", tag="obig", bufs=2)
        for ic in range(CH):
            o_ps = ps.tile([P, 512], F32, name=f"ops{ic}", tag=f"{bank}{ic}")
            for cc in range(CH):
                nc.tensor.matmul(
                    o_ps,
                    lhsT=mt_sb[cc][:, ic * P : (ic + 1) * P],
                    rhs=x16_sb[cc][:, nt * 512 : (nt + 1) * 512],
                    start=cc == 0,
                    stop=cc == CH - 1,
                )
            out_evac(o_big[:, ic * 512 : (ic + 1) * 512], o_ps, ic, (nt + ic) % 2)
        q = nc.sync if nt % 2 == 0 else nc.scalar
        q.dma_start(out[:, nt * CH * 512 : (nt + 1) * CH * 512], o_big)

    nt = NG - 1
    o_big = so.tile([P, CH * 512], F16, name="obig", tag="obig", bufs=2)
    qs = [nc.sync, nc.scalar, nc.gpsimd, nc.sync]
    for ic in range(CH):
        for half in range(2):
            bank = "pa" if half == 0 else "pb"
            o_ps = ps.tile([P, 256], F32, name=f"opsl{ic}{half}", tag=f"{bank}{ic}")
            nlo = nt * 512 + half * 256
            for cc in range(CH):
                nc.tensor.matmul(
                    o_ps,
                    lhsT=mt_sb[cc][:, ic * P : (ic + 1) * P],
                    rhs=x16_sb[cc][:, nlo : nlo + 256],
                    start=cc == 0,
                    stop=cc == CH - 1,
                )
            out_evac(
                o_big[:, ic * 512 + half * 256 : ic * 512 + half * 256 + 256],
                o_ps, ic, (ic + half) % 2,
            )
        qs[ic].dma_start(
            out[:, nt * CH * 512 + ic * 512 : nt * CH * 512 + (ic + 1) * 512],
            o_big[:, ic * 512 : (ic + 1) * 512],
        )

    for pool in (so, sb, ps):
        pool.release()


def _build_nc(repeat=1):
    nc = bacc.Bacc(
        "TRN2",
        target_bir_lowering=False,
        debug=False,
        num_devices=B,
        enable_asserts=False,
    )
    io = {}
    dt = nc.dram_tensor
    # xt16: granule-contiguous X^T blob: row (g*P+p) = 4 tiles' channel rows
    io["xt16"] = dt("xt16", (NG * P, 4 * C), F16, kind="ExternalInput").ap()
    io["x16"] = dt("x16", (C, HW), F16, kind="ExternalInput").ap()
    # weights prearranged [P, CH*C] so each is one contiguous DMA
    io["wqt"] = dt("wqt", (P, CH * C), F32R, kind="ExternalInput").ap()
    io["wkt"] = dt("wkt", (P, CH * C), F32R, kind="ExternalInput").ap()
    io["wv16"] = dt("wv16", (P, CH * C), F16, kind="ExternalInput").ap()
    io["bqr"] = dt("bqr", (1, C), F32R, kind="ExternalInput").ap()
    io["bkr"] = dt("bkr", (1, C), F32R, kind="ExternalInput").ap()
    io["nbkr"] = dt("nbkr", (1, C), F32, kind="ExternalInput").ap()
    io["bv16"] = dt("bv16", (P, CH), F16, kind="ExternalInput").ap()
    io["id16"] = dt("id16", (P, P), F16, kind="ExternalInput").ap()
    io["id32"] = dt("id32", (P, P), F32R, kind="ExternalInput").ap()
    # out: [P, NT*CH*512] f16, host unscrambles
    io["out16"] = dt("out16", (P, NG * CH * 512), F16, kind="ExternalOutput").ap()
    with tile.TileContext(nc) as tc:
        for _ in range(repeat):
            _body(tc, nc, io)
    nc.compile()
    return nc


_NC_CACHE = None


def get_nc():
    global _NC_CACHE
    if _NC_CACHE is None:
        _NC_CACHE = _build_nc()
    return _NC_CACHE


def prep_in_maps(x, wq, bq, wk, bk, wv, bv):
    """Host-side input prep: reshape/transpose/dtype casts only."""
    x = np.asarray(x, dtype=np.float32)
    X = x.reshape(B, C, HW)
    XT = X.transpose(0, 2, 1)  # [B, N, C]
    # granule-contiguous blob: [B, NG*P, 4*C]
    xt16 = np.ascontiguousarray(
        XT.reshape(B, NG, 4, P, C).transpose(0, 1, 3, 2, 4).reshape(B, NG * P, 4 * C)
    ).astype(np.float16)
    x16 = X.astype(np.float16)

    def warr(w):  # [C, C] -> w^T chunked [P, CH*C]
        wt = np.asarray(w, np.float32).T  # [c, o]
        return np.ascontiguousarray(
            wt.reshape(CH, P, C).transpose(1, 0, 2).reshape(P, CH * C)
        )

    wqta = warr(wq)
    wkta = warr(wk)
    wv16a = np.ascontiguousarray(
        np.asarray(wv, np.float32).reshape(CH, P, C).transpose(1, 0, 2).reshape(P, CH * C)
    ).astype(np.float16)
    bqr = np.asarray(bq, np.float32).reshape(1, C)
    bkr = np.asarray(bk, np.float32).reshape(1, C)
    nbkr = (float(HW) * np.asarray(bk, np.float32)).reshape(1, C)
    bv16 = np.ascontiguousarray(
        np.asarray(bv, np.float32).reshape(CH, P).T
    ).astype(np.float16)
    id16 = np.eye(P, dtype=np.float16)
    id32 = np.eye(P, dtype=np.float32)
    in_maps = []
    for b in range(B):
        in_maps.append(
            {
                "xt16": xt16[b],
                "x16": np.ascontiguousarray(x16[b]),
                "wqt": wqta,
                "wkt": wkta,
                "wv16": wv16a,
                "bqr": bqr,
                "bkr": bkr,
                "nbkr": nbkr,
                "bv16": bv16,
                "id16": id16,
                "id32": id32,
            }
        )
    return in_maps


def kernel(x, wq, bq, wk, bk, wv, bv):
    nc = get_nc()
    in_maps = prep_in_maps(x, wq, bq, wk, bk, wv, bv)
    res = run_bass_kernel_spmd(nc, in_maps, core_ids=list(range(B)))
    # out16 blob [P, NG*CH*512]: [p, nt, e, n_local] -> out[e*P+p, nt*512+n]
    blobs = np.stack([res.results[b]["out16"] for b in range(B)])
    outs = blobs.reshape(B, P, NG, CH, 512).transpose(0, 3, 1, 2, 4).reshape(B, C, HW)
    return outs.reshape(B, C, 64, 64).astype(np.float32)
